# revision 23
# baseline (speedup 1.0000x reference)
"""Trainium2 Bass kernel for nn_Attention_64819646431478.

Single-layer causal attention, B=1, T=2048, DIM=1024, 16 heads, head_dim=64,
f32, with RMSNorm (eps=f32 eps) on Q and K heads.

Sharding: tensor-parallel over heads across 8 NeuronCores (2 heads/core).
Each core computes its heads' Q/K/V projections, causal attention, and the
partial output projection against its 128-row slice of w_o; the host sums
the 8 partial outputs (the "all-reduce" of the hint, done at gather time).

v2 layout/schedule notes:
  - All dram inputs are host-preswizzled so every DMA is a dense
    [128, n] copy (fast descriptor generation, early start); xT arrives
    in 32 (chunk-major) pieces so the first projection can begin after
    ~1MB instead of the full 4MB.
  - Prologue computes Q and K projections + RMS norms for all chunks
    using the Square/Sqrt ACT table set, then the main loop switches to
    the Exp set exactly once (table thrash costs 1.3us per switch).
  - Per-head 1/rms and 1/sum_exp broadcasts along partitions are done by
    gpsimd.partition_broadcast instead of PE matmuls; the normalize
    multiplies are fused scalar_tensor_tensor ops that also apply gamma.
  - Scores are computed transposed per 128-key tile: ST[tk, tq] = K@Q^T,
    exp on ACT per [128,512] tile with double-buffered PSUM so score
    matmuls never wait on exp; causal masks (bf16 multiply) only touch
    the 4 diagonal tiles per chunk.
  - The softmax denominator is free: V tiles carry a shared ones column
    (layout [V0 | 1 | V1]) so head0's PV matmul yields sums in row 64
    and head1's in row 0.
  - Output projection is emitted per (mu, chunk) with bf16 staging; the
    PSUM->SBUF copies alternate gpsimd/vector to keep DVE off the
    critical path, and the partial output is written in bf16 (host sums
    the 8 partials in f64).
"""

import os
import sys
import types

import numpy as np

# --- environment bootstrap (harness may run us from a bare directory) ---
for _p in ("/root/.axon_site", "/root/.axon_site/_ro/trn_rl_repo",
           "/root/.axon_site/_ro/pypackages", "/opt/trn_rl_repo"):
    if os.path.isdir(_p) and _p not in sys.path:
        sys.path.append(_p)


def _install_ntff_shim():
    """Provide antenv.axon_hooks (missing in this image) so trace=True works."""
    if "antenv.axon_hooks" in sys.modules:
        return
    mod = types.ModuleType("antenv.axon_hooks")
    mod._hook = None
    mod.set_axon_ntff_profile_hook = lambda h: setattr(mod, "_hook", h)
    mod.get_axon_ntff_profile_hook = lambda: mod._hook
    sys.modules["antenv.axon_hooks"] = mod
    try:
        import antenv
        antenv.axon_hooks = mod
        from trn_agent_boot.trn_boot import _ntff_profile_via_ctypes
        mod.set_axon_ntff_profile_hook(
            _ntff_profile_via_ctypes("/opt/axon/libaxon_pjrt.so"))
    except Exception:
        pass


_install_ntff_shim()

import ml_dtypes  # noqa: E402

import concourse.mybir as mybir  # noqa: E402
import concourse.tile as tile  # noqa: E402
from concourse import bacc  # noqa: E402

F32 = mybir.dt.float32
BF16 = mybir.dt.bfloat16
NP_BF16 = ml_dtypes.bfloat16
AF = mybir.ActivationFunctionType

T = 2048
C = 1024
D = 64
NCORES = 8
HPC = 2            # heads per core
JPC = HPC * D      # 128 j-columns per core
NTQ = 4            # tq chunks of 512
TQ = 512
NTK = 16           # tk tiles of 128
EPS = float(np.finfo(np.float32).eps)


def build_nc():
    nc = bacc.Bacc("TRN2", target_bir_lowering=False, debug=False,
                   num_devices=NCORES)

    xT_d = nc.dram_tensor("xT", [128, 8, T], BF16, kind="ExternalInput")
    wq_d = nc.dram_tensor("wq", [128, 8, 128], BF16, kind="ExternalInput")
    wk_d = nc.dram_tensor("wk", [128, 8, 128], BF16, kind="ExternalInput")
    wv_d = nc.dram_tensor("wv", [128, 8, 128], BF16, kind="ExternalInput")
    wo_d = nc.dram_tensor("wo", [128, C], BF16, kind="ExternalInput")
    masks_d = nc.dram_tensor("masks", [128, 4, TQ], BF16,
                             kind="ExternalInput")
    gq_d = nc.dram_tensor("gq", [128, 1], F32, kind="ExternalInput")
    gk_d = nc.dram_tensor("gk", [128, 1], F32, kind="ExternalInput")
    onescol_d = nc.dram_tensor("onescol", [128, 65], BF16,
                               kind="ExternalInput")
    bc_d = nc.dram_tensor("bc", [128, 128], BF16, kind="ExternalInput")
    ident_d = nc.dram_tensor("ident", [128, 128], BF16, kind="ExternalInput")
    outT_d = nc.dram_tensor("outT", [128, 8, T], BF16, kind="ExternalOutput")

    MUL = mybir.AluOpType.mult

    with tile.TileContext(nc) as tc, nc.allow_low_precision("bf16 kernel"):
        from contextlib import ExitStack
        with ExitStack() as ctx:
            consts = ctx.enter_context(tc.tile_pool(name="consts", bufs=1))
            acts = ctx.enter_context(tc.tile_pool(name="acts", bufs=1))

            wq_sb = consts.tile([128, 8, 128], BF16)
            wk_sb = consts.tile([128, 8, 128], BF16)
            wv_sb = consts.tile([128, 8, 128], BF16)
            wo_sb = consts.tile([128, C], BF16)
            msb = consts.tile([128, 4, TQ], BF16)
            gq_sb = consts.tile([128, 1], F32)
            gk_sb = consts.tile([128, 1], F32)
            ones2c = consts.tile([128, 65], BF16)
            bc_sb = consts.tile([128, 128], BF16)
            ident_sb = consts.tile([128, 128], BF16)
            eps_sb = consts.tile([65, 1], F32)

            nc.gpsimd.dma_start(out=wq_sb[:], in_=wq_d[:])
            nc.gpsimd.dma_start(out=wk_sb[:], in_=wk_d[:])
            nc.gpsimd.dma_start(out=ones2c[:], in_=onescol_d[:])
            nc.gpsimd.dma_start(out=bc_sb[:], in_=bc_d[:])
            nc.gpsimd.dma_start(out=gq_sb[:], in_=gq_d[:])
            nc.gpsimd.dma_start(out=gk_sb[:], in_=gk_d[:])
            nc.gpsimd.dma_start(out=ident_sb[:], in_=ident_d[:])
            nc.gpsimd.dma_start(out=wv_sb[:], in_=wv_d[:])
            nc.gpsimd.dma_start(out=msb[:], in_=masks_d[:])
            nc.gpsimd.dma_start(out=wo_sb[:], in_=wo_d[:])
            nc.vector.memset(eps_sb[:], EPS)

            xT_sb = acts.tile([128, 8, T], BF16)
            for c4 in range(NTQ):
                sl = slice(TQ * c4, TQ * (c4 + 1))
                for ci in range(8):
                    nc.sync.dma_start(out=xT_sb[:, ci, sl],
                                      in_=xT_d[:, ci, sl])

            # ---- persistent activations ----
            QTn = acts.tile([128, T], BF16)
            # per-head K^T zero-padded to 128 partitions: keeps every
            # matmul at the K=128 PE-array config (K=64/128 alternation
            # costs ~100ns per switch)
            KZ = [acts.tile([128, T], BF16, name=f"KZ{h}")
                  for h in range(HPC)]
            V_sb = acts.tile([128, NTK, 130], BF16)
            vview = V_sb[:].rearrange("p r (a b) -> p r a b", b=65)
            nc.vector.memset(vview[:, :, :, 64:65], 1.0)
            recq = acts.tile([128, T], BF16)
            reck = acts.tile([128, T], BF16)
            sg128 = acts.tile([128, T], BF16)
            nc.vector.memset(KZ[0][:], 0.0)
            nc.vector.memset(KZ[1][:], 0.0)
            nc.vector.memset(recq[:], 0.0)
            nc.vector.memset(reck[:], 0.0)
            nc.vector.memset(sg128[:], 0.0)
            rms_q = acts.tile([65, T], F32)
            rms_k = acts.tile([65, T], F32)
            rrf_q = acts.tile([65, T], F32)
            rrf_k = acts.tile([65, T], F32)
            sg = [acts.tile([1, T], F32, name=f"sg{h}") for h in range(HPC)]
            sgf = [acts.tile([1, T], F32, name=f"sgf{h}")
                   for h in range(HPC)]

            # ============ Phase B: projections + RMS norms + V ============
            with (
                tc.tile_pool(name="rawp", bufs=3) as rawp,
                tc.tile_pool(name="sqp", bufs=2) as sqp,
                tc.tile_pool(name="vtp", bufs=2) as vtp,
                tc.tile_pool(name="ps_pp", bufs=3, space="PSUM") as ps_pp,
                tc.tile_pool(name="ps_sm", bufs=3, space="PSUM") as ps_sm,
                tc.tile_pool(name="ps_tp", bufs=2, space="PSUM") as ps_tp,
            ):
                for c4 in range(NTQ):
                    sl = slice(TQ * c4, TQ * (c4 + 1))
                    for w_sb, g_sb, rms_sb, rrf_sb, rec_sb, qk in (
                        (wq_sb, gq_sb, rms_q, rrf_q, recq, "q"),
                        (wk_sb, gk_sb, rms_k, rrf_k, reck, "k"),
                    ):
                        pp = ps_pp.tile([128, TQ], F32, tag="pp", name="pp")
                        for ci in range(8):
                            nc.tensor.matmul(
                                pp[:], w_sb[:, ci, :], xT_sb[:, ci, sl],
                                start=(ci == 0), stop=(ci == 7))
                        sq = sqp.tile([128, TQ], BF16, tag="sq", name="sq")
                        nc.scalar.activation(sq[:], pp[:], AF.Square)
                        raw = rawp.tile([128, TQ], BF16, tag="raw",
                                        name="raw")
                        nc.scalar.activation(raw[:], pp[:], AF.Copy)
                        sums = ps_sm.tile([65, TQ], F32, tag="sm",
                                          name="sums")
                        nc.tensor.matmul(sums[:], ones2c[:], sq[:],
                                         start=True, stop=True)
                        nc.scalar.activation(rms_sb[:, sl], sums[:],
                                             AF.Sqrt, bias=eps_sb[:],
                                             scale=1.0 / D)
                        nc.vector.reciprocal_approx_fast(
                            out=rrf_sb[:, sl], in_=rms_sb[:, sl])
                        for h in range(HPC):
                            nc.vector.tensor_copy(
                                rec_sb[64 * h:64 * h + 1, sl],
                                rrf_sb[64 * h:64 * h + 1, sl])
                        bb = ps_sm.tile([128, TQ], F32, tag="sm",
                                        name="bb")
                        nc.tensor.matmul(bb[:], bc_sb[:], rec_sb[:, sl],
                                         start=True, stop=True)
                        if qk == "q":
                            nc.vector.scalar_tensor_tensor(
                                out=QTn[:, sl], in0=raw[:],
                                scalar=g_sb[:], in1=bb[:],
                                op0=MUL, op1=MUL)
                        else:
                            for h in range(HPC):
                                hsl = slice(64 * h, 64 * (h + 1))
                                nc.vector.scalar_tensor_tensor(
                                    out=KZ[h][hsl, sl], in0=raw[hsl, :],
                                    scalar=g_sb[hsl, :], in1=bb[hsl, :],
                                    op0=MUL, op1=MUL)

                # V projections last: pure PE/DVE work (no ACT chains), so
                # the final QK-norm chain latency hides under it
                for c4 in range(NTQ):
                    sl = slice(TQ * c4, TQ * (c4 + 1))
                    pv = ps_pp.tile([128, TQ], F32, tag="pp",
                                    name=f"pv{c4}")
                    for ci in range(8):
                        nc.tensor.matmul(
                            pv[:], wv_sb[:, ci, :], xT_sb[:, ci, sl],
                            start=(ci == 0), stop=(ci == 7))
                    vt = vtp.tile([128, TQ], BF16, tag="vt", name=f"vt{c4}")
                    nc.scalar.activation(vt[:], pv[:], AF.Copy)
                    for rl in range(4):
                        r = 4 * c4 + rl
                        tp = ps_tp.tile([128, 128], BF16, tag="tp",
                                        name=f"tp{r}")
                        nc.tensor.transpose(
                            tp[:], vt[:, 128 * rl:128 * (rl + 1)],
                            ident_sb[:])
                        dst_v = V_sb[:, r, :].rearrange(
                            "p (a b) -> p a b", b=65)[:, :, 0:64]
                        src_v = tp[:].rearrange("p (a b) -> p a b", b=64)
                        nc.vector.tensor_copy(dst_v, src_v)

            # ========== Phase C: attention + normalize + w_o ==========
            with (
                tc.tile_pool(name="ep", bufs=4) as ep,
                tc.tile_pool(name="ctxp", bufs=2) as ctxp,
                tc.tile_pool(name="b2p", bufs=2) as b2p,
                tc.tile_pool(name="stgp", bufs=4) as stgp,
                tc.tile_pool(name="ps_st", bufs=2, space="PSUM") as ps_st,
                tc.tile_pool(name="ps_ot", bufs=1, space="PSUM") as ps_ot,
                tc.tile_pool(name="ps_wk", bufs=2, space="PSUM") as ps_wk,
            ):
                for c4 in range(NTQ):
                    sl = slice(TQ * c4, TQ * (c4 + 1))
                    n_tk = 4 * (c4 + 1)
                    ot = [ps_ot.tile([65, TQ], F32, tag=f"ot{h}",
                                     name=f"ot{h}_{c4}")
                          for h in range(HPC)]
                    groups = [(g, h) for g in range(n_tk // 2)
                              for h in range(HPC)]

                    def emit_pv(g, h, e_t):
                        for j in range(2):
                            r = 2 * g + j
                            nc.tensor.matmul(
                                ot[h][:], V_sb[:, r, 65 * h:65 * h + 65],
                                e_t[:, TQ * j:TQ * (j + 1)],
                                start=(r == 0), stop=(r == n_tk - 1))

                    prev = None
                    for g, h in groups:
                        st = ps_st.tile([128, 2 * TQ], F32, tag="st",
                                        name="st")
                        for j in range(2):
                            r = 2 * g + j
                            nc.tensor.matmul(
                                st[:, TQ * j:TQ * (j + 1)],
                                KZ[h][:, 128 * r:128 * (r + 1)],
                                QTn[:, sl], start=True, stop=True)
                        e_t = ep.tile([128, 2 * TQ], BF16, tag="e",
                                      name="e")
                        nc.scalar.activation(e_t[:], st[:], AF.Exp,
                                             scale=float(D) ** -0.5)
                        s0 = 2 * g - 4 * c4
                        if s0 >= 0:
                            ev = e_t[:].rearrange("p (s f) -> p s f", f=TQ)
                            nc.vector.tensor_mul(ev, ev,
                                                 msb[:, s0:s0 + 2, :])
                        if prev is not None:
                            emit_pv(*prev)
                        prev = (g, h, e_t)
                    emit_pv(*prev)

                    # softmax denominators -> 1/sum -> broadcast;
                    # the last chunk runs in 256-wide halves to halve the
                    # exposed tail latency
                    halves = (((0, TQ),) if c4 < 3
                              else ((0, TQ // 2), (TQ // 2, TQ)))
                    ctxT = ctxp.tile([128, TQ], BF16, tag="ctx",
                                     name=f"ctx{c4}")
                    b2s = b2p.tile([128, TQ], BF16, tag="b2",
                                   name=f"b2s{c4}")
                    for hv, (lo, hi) in enumerate(halves):
                        hsl2 = slice(TQ * c4 + lo, TQ * c4 + hi)
                        for h in range(HPC):
                            nc.vector.tensor_copy(sg[h][0:1, hsl2],
                                                  ot[h][64:65, lo:hi])
                            nc.vector.reciprocal_approx_fast(
                                out=sgf[h][0:1, hsl2],
                                in_=sg[h][0:1, hsl2])
                            nc.vector.tensor_copy(
                                sg128[64 * h:64 * h + 1, hsl2],
                                sgf[h][0:1, hsl2])
                        b2 = ps_wk.tile([128, TQ], F32, tag="wk",
                                        name=f"b2{c4}_{hv}")
                        nc.tensor.matmul(b2[:, lo:hi], bc_sb[:],
                                         sg128[:, hsl2],
                                         start=True, stop=True)
                        nc.vector.tensor_copy(b2s[:, lo:hi], b2[:, lo:hi])
                        for h in range(HPC):
                            nc.vector.scalar_tensor_tensor(
                                out=ctxT[64 * h:64 * (h + 1), lo:hi],
                                in0=ot[h][0:64, lo:hi], scalar=1.0,
                                in1=b2s[64 * h:64 * (h + 1), lo:hi],
                                op0=MUL, op1=MUL)
                        # output projection; bf16 staging split DVE/ACT
                        for mu in range(8):
                            wop = ps_wk.tile([128, TQ], F32, tag="wk",
                                             name=f"wop{mu}_{c4}_{hv}")
                            nc.tensor.matmul(
                                wop[:, lo:hi],
                                wo_sb[:, 128 * mu:128 * (mu + 1)],
                                ctxT[:, lo:hi], start=True, stop=True)
                            stg = stgp.tile([128, TQ], BF16, tag="stg",
                                            name=f"stg{mu}_{c4}_{hv}")
                            if c4 == 3 and mu % 2 == 1:
                                nc.scalar.activation(stg[:, lo:hi],
                                                     wop[:, lo:hi],
                                                     AF.Copy)
                            else:
                                nc.vector.tensor_copy(stg[:, lo:hi],
                                                      wop[:, lo:hi])
                            nc.sync.dma_start(
                                out=outT_d[:, mu, hsl2],
                                in_=stg[:, lo:hi])

    nc.compile()
    return nc


_NC_CACHE = None


def _get_nc():
    global _NC_CACHE
    if _NC_CACHE is None:
        _NC_CACHE = build_nc()
    return _NC_CACHE


def _make_in_maps(x, w_q, w_k, w_v, w_o, q_gamma, k_gamma):
    x = np.asarray(x, dtype=np.float32).reshape(T, C)
    # xT[p, c, t] = x[t, c*128+p]
    xT = np.ascontiguousarray(
        x.reshape(T, 8, 128).transpose(2, 1, 0)).astype(NP_BF16)

    p = np.arange(128)
    f = np.arange(TQ)
    masks = np.zeros((128, 4, TQ), dtype=NP_BF16)
    for s in range(4):
        masks[:, s, :] = (f[None, :] >= (p[:, None] + 128 * s)).astype(
            NP_BF16)

    gq = np.tile(np.asarray(q_gamma, np.float32), 2).reshape(128, 1)
    gk = np.tile(np.asarray(k_gamma, np.float32), 2).reshape(128, 1)
    onescol = np.zeros((128, 65), dtype=NP_BF16)
    onescol[0:64, 0] = 1
    onescol[64:128, 64] = 1
    # broadcast stationary: row 0 -> out partitions 0..63,
    # row 64 -> out partitions 64..127
    bc = np.zeros((128, 128), dtype=NP_BF16)
    bc[0, 0:64] = 1
    bc[64, 64:128] = 1

    ident = np.eye(128, dtype=NP_BF16)
    common = dict(xT=xT, masks=masks, gq=gq, gk=gk, onescol=onescol, bc=bc,
                  ident=ident)

    in_maps = []
    for i in range(NCORES):
        rows = slice(JPC * i, JPC * (i + 1))

        def wsw(w):
            # [p, c, j] = W[rows][j, c*128+p]
            W = np.asarray(w, np.float32)[rows]           # [128, C]
            return np.ascontiguousarray(
                W.reshape(128, 8, 128).transpose(2, 1, 0)).astype(NP_BF16)

        wo = np.asarray(w_o, np.float32)[:, rows].T        # [128, C]
        in_maps.append(dict(common, wq=wsw(w_q), wk=wsw(w_k), wv=wsw(w_v),
                            wo=np.ascontiguousarray(wo).astype(NP_BF16)))
    return in_maps


def _run(x, w_q, w_k, w_v, w_o, q_gamma, k_gamma, trace=False):
    import time

    from concourse.bass_utils import run_bass_kernel_spmd
    nc = _get_nc()
    in_maps = _make_in_maps(x, w_q, w_k, w_v, w_o, q_gamma, k_gamma)
    res = None
    for attempt in range(3):
        try:
            res = run_bass_kernel_spmd(nc, in_maps, list(range(NCORES)),
                                       trace=trace)
            break
        except Exception:
            # rare transient NRT_EXEC_UNIT_UNRECOVERABLE under axon; the
            # terminal resets the device on the next load
            if attempt == 2:
                raise
            time.sleep(3.0)
    acc = np.zeros((128, 8, T), dtype=np.float64)
    for r in res.results:
        acc += r["outT"].astype(np.float64)
    # out[t, m*128+p] = acc[p, m, t]
    out = acc.transpose(2, 1, 0).reshape(T, C).astype(np.float32)
    return out.reshape(1, T, C), res


def kernel(x, w_q, w_k, w_v, w_o, q_gamma, k_gamma):
    out, _ = _run(x, w_q, w_k, w_v, w_o, q_gamma, k_gamma, trace=False)
    return out


# revision 24
# speedup vs baseline: 1.1561x; 1.1561x over previous
"""Trainium2 Bass kernel for nn_Attention_64819646431478.

Single-layer causal attention, B=1, T=2048, DIM=1024, 16 heads, head_dim=64,
f32, with RMSNorm (eps=f32 eps) on Q and K heads.

Sharding: tensor-parallel over heads across 8 NeuronCores (2 heads/core).
Each core computes its heads' Q/K/V projections, causal attention, and the
partial output projection against its 128-row slice of w_o; the host sums
the 8 partial outputs (the "all-reduce" of the hint, done at gather time).

v2 layout/schedule notes:
  - All dram inputs are host-preswizzled so every DMA is a dense
    [128, n] copy (fast descriptor generation, early start); xT arrives
    in 32 (chunk-major) pieces so the first projection can begin after
    ~1MB instead of the full 4MB.
  - Prologue computes Q and K projections + RMS norms for all chunks
    using the Square/Sqrt ACT table set, then the main loop switches to
    the Exp set exactly once (table thrash costs 1.3us per switch).
  - Per-head 1/rms and 1/sum_exp broadcasts along partitions are done by
    gpsimd.partition_broadcast instead of PE matmuls; the normalize
    multiplies are fused scalar_tensor_tensor ops that also apply gamma.
  - Scores are computed transposed per 128-key tile: ST[tk, tq] = K@Q^T,
    exp on ACT per [128,512] tile with double-buffered PSUM so score
    matmuls never wait on exp; causal masks (bf16 multiply) only touch
    the 4 diagonal tiles per chunk.
  - The softmax denominator is free: V tiles carry a shared ones column
    (layout [V0 | 1 | V1]) so head0's PV matmul yields sums in row 64
    and head1's in row 0.
  - Output projection is emitted per (mu, chunk) with bf16 staging; the
    PSUM->SBUF copies alternate gpsimd/vector to keep DVE off the
    critical path, and the partial output is written in bf16 (host sums
    the 8 partials in f64).
"""

import os
import sys
import types

import numpy as np

# --- environment bootstrap (harness may run us from a bare directory) ---
for _p in ("/root/.axon_site", "/root/.axon_site/_ro/trn_rl_repo",
           "/root/.axon_site/_ro/pypackages", "/opt/trn_rl_repo"):
    if os.path.isdir(_p) and _p not in sys.path:
        sys.path.append(_p)


def _install_ntff_shim():
    """Provide antenv.axon_hooks (missing in this image) so trace=True works."""
    if "antenv.axon_hooks" in sys.modules:
        return
    mod = types.ModuleType("antenv.axon_hooks")
    mod._hook = None
    mod.set_axon_ntff_profile_hook = lambda h: setattr(mod, "_hook", h)
    mod.get_axon_ntff_profile_hook = lambda: mod._hook
    sys.modules["antenv.axon_hooks"] = mod
    try:
        import antenv
        antenv.axon_hooks = mod
        from trn_agent_boot.trn_boot import _ntff_profile_via_ctypes
        mod.set_axon_ntff_profile_hook(
            _ntff_profile_via_ctypes("/opt/axon/libaxon_pjrt.so"))
    except Exception:
        pass


_install_ntff_shim()

import ml_dtypes  # noqa: E402

import concourse.mybir as mybir  # noqa: E402
import concourse.tile as tile  # noqa: E402
from concourse import bacc  # noqa: E402

F32 = mybir.dt.float32
BF16 = mybir.dt.bfloat16
NP_BF16 = ml_dtypes.bfloat16
AF = mybir.ActivationFunctionType

T = 2048
C = 1024
D = 64
NCORES = 8
HPC = 2            # heads per core
JPC = HPC * D      # 128 j-columns per core
NTQ = 4            # tq chunks of 512
TQ = 512
NTK = 16           # tk tiles of 128
EPS = float(np.finfo(np.float32).eps)


def build_nc():
    nc = bacc.Bacc("TRN2", target_bir_lowering=False, debug=False,
                   num_devices=NCORES)

    xT_d = nc.dram_tensor("xT", [128, 8, T], BF16, kind="ExternalInput")
    wq_d = nc.dram_tensor("wq", [128, 8, 128], BF16, kind="ExternalInput")
    wk_d = nc.dram_tensor("wk", [128, 8, 128], BF16, kind="ExternalInput")
    wv_d = nc.dram_tensor("wv", [128, 8, 128], BF16, kind="ExternalInput")
    wo_d = nc.dram_tensor("wo", [128, C], BF16, kind="ExternalInput")
    masks_d = nc.dram_tensor("masks", [128, 4, TQ], BF16,
                             kind="ExternalInput")
    gq_d = nc.dram_tensor("gq", [128, 1], F32, kind="ExternalInput")
    gk_d = nc.dram_tensor("gk", [128, 1], F32, kind="ExternalInput")
    onescol_d = nc.dram_tensor("onescol", [128, 65], BF16,
                               kind="ExternalInput")
    bc_d = nc.dram_tensor("bc", [128, 128], BF16, kind="ExternalInput")
    ident_d = nc.dram_tensor("ident", [128, 128], BF16, kind="ExternalInput")
    outT_d = nc.dram_tensor("outT", [128, 8, T], BF16, kind="ExternalOutput")

    MUL = mybir.AluOpType.mult

    with tile.TileContext(nc) as tc, nc.allow_low_precision("bf16 kernel"):
        from contextlib import ExitStack
        with ExitStack() as ctx:
            consts = ctx.enter_context(tc.tile_pool(name="consts", bufs=1))
            acts = ctx.enter_context(tc.tile_pool(name="acts", bufs=1))

            wq_sb = consts.tile([128, 8, 128], BF16)
            wk_sb = consts.tile([128, 8, 128], BF16)
            wv_sb = consts.tile([128, 8, 128], BF16)
            wo_sb = consts.tile([128, C], BF16)
            msb = consts.tile([128, 4, TQ], BF16)
            gq_sb = consts.tile([128, 1], F32)
            gk_sb = consts.tile([128, 1], F32)
            ones2c = consts.tile([128, 65], BF16)
            bc_sb = consts.tile([128, 128], BF16)
            ident_sb = consts.tile([128, 128], BF16)
            eps_sb = consts.tile([65, 1], F32)

            nc.gpsimd.dma_start(out=wq_sb[:], in_=wq_d[:])
            nc.gpsimd.dma_start(out=wk_sb[:], in_=wk_d[:])
            nc.gpsimd.dma_start(out=ones2c[:], in_=onescol_d[:])
            nc.gpsimd.dma_start(out=bc_sb[:], in_=bc_d[:])
            nc.gpsimd.dma_start(out=gq_sb[:], in_=gq_d[:])
            nc.gpsimd.dma_start(out=gk_sb[:], in_=gk_d[:])
            nc.gpsimd.dma_start(out=ident_sb[:], in_=ident_d[:])
            nc.gpsimd.dma_start(out=wv_sb[:], in_=wv_d[:])
            nc.gpsimd.dma_start(out=msb[:], in_=masks_d[:])
            nc.gpsimd.dma_start(out=wo_sb[:], in_=wo_d[:])
            nc.vector.memset(eps_sb[:], EPS)

            xT_sb = acts.tile([128, 8, T], BF16)
            for c4 in range(NTQ):
                sl = slice(TQ * c4, TQ * (c4 + 1))
                for ci in range(8):
                    nc.sync.dma_start(out=xT_sb[:, ci, sl],
                                      in_=xT_d[:, ci, sl])

            # ---- persistent activations ----
            QTn = acts.tile([128, T], BF16)
            # per-head K^T zero-padded to 128 partitions: keeps every
            # matmul at the K=128 PE-array config (K=64/128 alternation
            # costs ~100ns per switch)
            KZ = [acts.tile([128, T], BF16, name=f"KZ{h}")
                  for h in range(HPC)]
            V_sb = acts.tile([128, NTK, 130], BF16)
            vview = V_sb[:].rearrange("p r (a b) -> p r a b", b=65)
            nc.vector.memset(vview[:, :, :, 64:65], 1.0)
            recq = acts.tile([128, T], BF16)
            reck = acts.tile([128, T], BF16)
            sg128 = acts.tile([128, T], BF16)
            nc.vector.memset(KZ[0][:], 0.0)
            nc.vector.memset(KZ[1][:], 0.0)
            nc.vector.memset(recq[:], 0.0)
            nc.vector.memset(reck[:], 0.0)
            nc.vector.memset(sg128[:], 0.0)
            rms_q = acts.tile([65, T], F32)
            rms_k = acts.tile([65, T], F32)
            rrf_q = acts.tile([65, T], F32)
            rrf_k = acts.tile([65, T], F32)
            sg = [acts.tile([1, T], F32, name=f"sg{h}") for h in range(HPC)]
            sgf = [acts.tile([1, T], F32, name=f"sgf{h}")
                   for h in range(HPC)]

            # ============ Phase B: projections + RMS norms + V ============
            with (
                tc.tile_pool(name="rawp", bufs=3) as rawp,
                tc.tile_pool(name="sqp", bufs=2) as sqp,
                tc.tile_pool(name="vtp", bufs=2) as vtp,
                tc.tile_pool(name="ps_pp", bufs=3, space="PSUM") as ps_pp,
                tc.tile_pool(name="ps_sm", bufs=3, space="PSUM") as ps_sm,
                tc.tile_pool(name="ps_tp", bufs=2, space="PSUM") as ps_tp,
            ):
                for c4 in range(NTQ):
                    sl = slice(TQ * c4, TQ * (c4 + 1))
                    for w_sb, g_sb, rms_sb, rrf_sb, rec_sb, qk in (
                        (wq_sb, gq_sb, rms_q, rrf_q, recq, "q"),
                        (wk_sb, gk_sb, rms_k, rrf_k, reck, "k"),
                    ):
                        pp = ps_pp.tile([128, TQ], F32, tag="pp", name="pp")
                        for ci in range(8):
                            nc.tensor.matmul(
                                pp[:], w_sb[:, ci, :], xT_sb[:, ci, sl],
                                start=(ci == 0), stop=(ci == 7))
                        sq = sqp.tile([128, TQ], BF16, tag="sq", name="sq")
                        nc.scalar.activation(sq[:], pp[:], AF.Square)
                        raw = rawp.tile([128, TQ], BF16, tag="raw",
                                        name="raw")
                        nc.vector.tensor_copy(raw[:], pp[:])
                        sums = ps_sm.tile([65, TQ], F32, tag="sm",
                                          name="sums")
                        nc.tensor.matmul(sums[:], ones2c[:], sq[:],
                                         start=True, stop=True)
                        nc.scalar.activation(rms_sb[:, sl], sums[:],
                                             AF.Sqrt, bias=eps_sb[:],
                                             scale=1.0 / D)
                        nc.vector.reciprocal_approx_fast(
                            out=rrf_sb[:, sl], in_=rms_sb[:, sl])
                        for h in range(HPC):
                            nc.vector.tensor_copy(
                                rec_sb[64 * h:64 * h + 1, sl],
                                rrf_sb[64 * h:64 * h + 1, sl])
                        bb = ps_sm.tile([128, TQ], F32, tag="sm",
                                        name="bb")
                        nc.tensor.matmul(bb[:], bc_sb[:], rec_sb[:, sl],
                                         start=True, stop=True)
                        if qk == "q":
                            nc.vector.scalar_tensor_tensor(
                                out=QTn[:, sl], in0=raw[:],
                                scalar=g_sb[:], in1=bb[:],
                                op0=MUL, op1=MUL)
                        else:
                            for h in range(HPC):
                                hsl = slice(64 * h, 64 * (h + 1))
                                nc.vector.scalar_tensor_tensor(
                                    out=KZ[h][hsl, sl], in0=raw[hsl, :],
                                    scalar=g_sb[hsl, :], in1=bb[hsl, :],
                                    op0=MUL, op1=MUL)

                    # V projection + PE transpose into V_sb
                    pv = ps_pp.tile([128, TQ], F32, tag="pp",
                                    name=f"pv{c4}")
                    for ci in range(8):
                        nc.tensor.matmul(
                            pv[:], wv_sb[:, ci, :], xT_sb[:, ci, sl],
                            start=(ci == 0), stop=(ci == 7))
                    vt = vtp.tile([128, TQ], BF16, tag="vt", name=f"vt{c4}")
                    nc.vector.tensor_copy(vt[:], pv[:])
                    for rl in range(4):
                        r = 4 * c4 + rl
                        tp = ps_tp.tile([128, 128], BF16, tag="tp",
                                        name=f"tp{r}")
                        nc.tensor.transpose(
                            tp[:], vt[:, 128 * rl:128 * (rl + 1)],
                            ident_sb[:])
                        dst_v = V_sb[:, r, :].rearrange(
                            "p (a b) -> p a b", b=65)[:, :, 0:64]
                        src_v = tp[:].rearrange("p (a b) -> p a b", b=64)
                        nc.vector.tensor_copy(dst_v, src_v)

            # ========== Phase C: attention + normalize + w_o ==========
            with (
                tc.tile_pool(name="ep", bufs=4) as ep,
                tc.tile_pool(name="ctxp", bufs=2) as ctxp,
                tc.tile_pool(name="b2p", bufs=2) as b2p,
                tc.tile_pool(name="stgp", bufs=4) as stgp,
                tc.tile_pool(name="ps_st", bufs=2, space="PSUM") as ps_st,
                tc.tile_pool(name="ps_ot", bufs=1, space="PSUM") as ps_ot,
                tc.tile_pool(name="ps_wk", bufs=2, space="PSUM") as ps_wk,
            ):
                for c4 in range(NTQ):
                    sl = slice(TQ * c4, TQ * (c4 + 1))
                    n_tk = 4 * (c4 + 1)
                    ot = [ps_ot.tile([65, TQ], F32, tag=f"ot{h}",
                                     name=f"ot{h}_{c4}")
                          for h in range(HPC)]
                    groups = [(g, h) for g in range(n_tk // 2)
                              for h in range(HPC)]

                    def emit_pv(g, h, e_t):
                        for j in range(2):
                            r = 2 * g + j
                            nc.tensor.matmul(
                                ot[h][:], V_sb[:, r, 65 * h:65 * h + 65],
                                e_t[:, TQ * j:TQ * (j + 1)],
                                start=(r == 0), stop=(r == n_tk - 1))

                    prev = None
                    for g, h in groups:
                        st = ps_st.tile([128, 2 * TQ], F32, tag="st",
                                        name="st")
                        for j in range(2):
                            r = 2 * g + j
                            nc.tensor.matmul(
                                st[:, TQ * j:TQ * (j + 1)],
                                KZ[h][:, 128 * r:128 * (r + 1)],
                                QTn[:, sl], start=True, stop=True)
                        e_t = ep.tile([128, 2 * TQ], BF16, tag="e",
                                      name="e")
                        nc.scalar.activation(e_t[:], st[:], AF.Exp,
                                             scale=float(D) ** -0.5)
                        s0 = 2 * g - 4 * c4
                        if s0 >= 0:
                            ev = e_t[:].rearrange("p (s f) -> p s f", f=TQ)
                            nc.vector.tensor_mul(ev, ev,
                                                 msb[:, s0:s0 + 2, :])
                        if prev is not None:
                            emit_pv(*prev)
                        prev = (g, h, e_t)
                    emit_pv(*prev)

                    # softmax denominators -> 1/sum -> broadcast;
                    # the last chunk runs in 256-wide halves to halve the
                    # exposed tail latency
                    halves = (((0, TQ),) if c4 < 3
                              else ((0, TQ // 2), (TQ // 2, TQ)))
                    ctxT = ctxp.tile([128, TQ], BF16, tag="ctx",
                                     name=f"ctx{c4}")
                    b2s = b2p.tile([128, TQ], BF16, tag="b2",
                                   name=f"b2s{c4}")
                    for hv, (lo, hi) in enumerate(halves):
                        hsl2 = slice(TQ * c4 + lo, TQ * c4 + hi)
                        for h in range(HPC):
                            nc.vector.tensor_copy(sg[h][0:1, hsl2],
                                                  ot[h][64:65, lo:hi])
                            nc.vector.reciprocal_approx_fast(
                                out=sgf[h][0:1, hsl2],
                                in_=sg[h][0:1, hsl2])
                            nc.vector.tensor_copy(
                                sg128[64 * h:64 * h + 1, hsl2],
                                sgf[h][0:1, hsl2])
                        b2 = ps_wk.tile([128, TQ], F32, tag="wk",
                                        name=f"b2{c4}_{hv}")
                        nc.tensor.matmul(b2[:, lo:hi], bc_sb[:],
                                         sg128[:, hsl2],
                                         start=True, stop=True)
                        nc.vector.tensor_copy(b2s[:, lo:hi], b2[:, lo:hi])
                        for h in range(HPC):
                            nc.vector.scalar_tensor_tensor(
                                out=ctxT[64 * h:64 * (h + 1), lo:hi],
                                in0=ot[h][0:64, lo:hi], scalar=1.0,
                                in1=b2s[64 * h:64 * (h + 1), lo:hi],
                                op0=MUL, op1=MUL)
                        # output projection; bf16 staging split DVE/ACT
                        for mu in range(8):
                            wop = ps_wk.tile([128, TQ], F32, tag="wk",
                                             name=f"wop{mu}_{c4}_{hv}")
                            nc.tensor.matmul(
                                wop[:, lo:hi],
                                wo_sb[:, 128 * mu:128 * (mu + 1)],
                                ctxT[:, lo:hi], start=True, stop=True)
                            stg = stgp.tile([128, TQ], BF16, tag="stg",
                                            name=f"stg{mu}_{c4}_{hv}")
                            if c4 == 3 and mu % 2 == 1:
                                nc.scalar.activation(stg[:, lo:hi],
                                                     wop[:, lo:hi],
                                                     AF.Copy)
                            else:
                                nc.vector.tensor_copy(stg[:, lo:hi],
                                                      wop[:, lo:hi])
                            nc.sync.dma_start(
                                out=outT_d[:, mu, hsl2],
                                in_=stg[:, lo:hi])

    nc.compile()
    return nc


_NC_CACHE = None


def _get_nc():
    global _NC_CACHE
    if _NC_CACHE is None:
        _NC_CACHE = build_nc()
    return _NC_CACHE


def _make_in_maps(x, w_q, w_k, w_v, w_o, q_gamma, k_gamma):
    x = np.asarray(x, dtype=np.float32).reshape(T, C)
    # xT[p, c, t] = x[t, c*128+p]
    xT = np.ascontiguousarray(
        x.reshape(T, 8, 128).transpose(2, 1, 0)).astype(NP_BF16)

    p = np.arange(128)
    f = np.arange(TQ)
    masks = np.zeros((128, 4, TQ), dtype=NP_BF16)
    for s in range(4):
        masks[:, s, :] = (f[None, :] >= (p[:, None] + 128 * s)).astype(
            NP_BF16)

    gq = np.tile(np.asarray(q_gamma, np.float32), 2).reshape(128, 1)
    gk = np.tile(np.asarray(k_gamma, np.float32), 2).reshape(128, 1)
    onescol = np.zeros((128, 65), dtype=NP_BF16)
    onescol[0:64, 0] = 1
    onescol[64:128, 64] = 1
    # broadcast stationary: row 0 -> out partitions 0..63,
    # row 64 -> out partitions 64..127
    bc = np.zeros((128, 128), dtype=NP_BF16)
    bc[0, 0:64] = 1
    bc[64, 64:128] = 1

    ident = np.eye(128, dtype=NP_BF16)
    common = dict(xT=xT, masks=masks, gq=gq, gk=gk, onescol=onescol, bc=bc,
                  ident=ident)

    in_maps = []
    for i in range(NCORES):
        rows = slice(JPC * i, JPC * (i + 1))

        def wsw(w):
            # [p, c, j] = W[rows][j, c*128+p]
            W = np.asarray(w, np.float32)[rows]           # [128, C]
            return np.ascontiguousarray(
                W.reshape(128, 8, 128).transpose(2, 1, 0)).astype(NP_BF16)

        wo = np.asarray(w_o, np.float32)[:, rows].T        # [128, C]
        in_maps.append(dict(common, wq=wsw(w_q), wk=wsw(w_k), wv=wsw(w_v),
                            wo=np.ascontiguousarray(wo).astype(NP_BF16)))
    return in_maps


def _run(x, w_q, w_k, w_v, w_o, q_gamma, k_gamma, trace=False):
    import time

    from concourse.bass_utils import run_bass_kernel_spmd
    nc = _get_nc()
    in_maps = _make_in_maps(x, w_q, w_k, w_v, w_o, q_gamma, k_gamma)
    res = None
    for attempt in range(3):
        try:
            res = run_bass_kernel_spmd(nc, in_maps, list(range(NCORES)),
                                       trace=trace)
            break
        except Exception:
            # rare transient NRT_EXEC_UNIT_UNRECOVERABLE under axon; the
            # terminal resets the device on the next load
            if attempt == 2:
                raise
            time.sleep(3.0)
    acc = np.zeros((128, 8, T), dtype=np.float64)
    for r in res.results:
        acc += r["outT"].astype(np.float64)
    # out[t, m*128+p] = acc[p, m, t]
    out = acc.transpose(2, 1, 0).reshape(T, C).astype(np.float32)
    return out.reshape(1, T, C), res


def kernel(x, w_q, w_k, w_v, w_o, q_gamma, k_gamma):
    out, _ = _run(x, w_q, w_k, w_v, w_o, q_gamma, k_gamma, trace=False)
    return out
